# revision 1
# baseline (speedup 1.0000x reference)
"""Batched GAT layer (B=8, N=2048, Fin=256, Fout=128) on 8 Trainium2 NeuronCores.

Strategy: data-parallel over batch B — one batch element per core. Inside
each core a column-block formulation keeps the softmax contraction (over
neighbors j) on the PSUM accumulation path of the tensor engine:

  h      = x @ W.T + b                      (PE, fp32)
  e[j,i] = leakyrelu(s1[i] + s2[j])         s1 = h a1, s2 = h a2
  p      = exp(e + maskbias)                maskbias = 0 / -240 (fp8 from host)
  out    = elu((p.T scaled) ... )           h'T[o,i] = sum_j h[j,o] p[j,i] / S[i]

Host-side work is layout only: transposes, dtype packing of adj into an
additive fp8 mask, and the final un-transpose of the per-core outputs.
"""
import numpy as np
import ml_dtypes

B, N, FIN, FOUT = 8, 2048, 256, 128
P = 128
NT = N // P          # 16 j-tiles
NC4 = N // 512       # 4 psum chunks
ALPHA = 0.4
MASK_NEG = -240.0

# j-tiles whose leakyrelu runs on the vector engine instead of ACT (load
# balance knob), and j-tiles whose mask-add runs on gpsimd instead of DVE.
DVE_LEAKY_TILES = frozenset({2, 5, 8, 11, 14})
GPS_EM_TILES = frozenset({3, 6, 9, 12, 15})

_cache = {}


def _build():
    import concourse.mybir as mybir
    import concourse.tile as tile
    from concourse import bacc
    from concourse.masks import make_identity

    F32 = mybir.dt.float32
    F32R = mybir.dt.float32r
    FP8 = mybir.dt.float8e4
    AF = mybir.ActivationFunctionType
    ALU = mybir.AluOpType

    nc = bacc.Bacc("TRN2", target_bir_lowering=False, debug=False)

    xT_d = nc.dram_tensor("xT", [FIN, N], F32, kind="ExternalInput").ap()
    adjm_d = nc.dram_tensor("adjm", [N, N], FP8, kind="ExternalInput").ap()
    wt_d = nc.dram_tensor("wt", [FIN, FOUT], F32, kind="ExternalInput").ap()
    bcol_d = nc.dram_tensor("bcol", [FOUT, 1], F32, kind="ExternalInput").ap()
    a1rep_d = nc.dram_tensor("a1rep", [FOUT, P], F32, kind="ExternalInput").ap()
    a2rep_d = nc.dram_tensor("a2rep", [P, FOUT], F32, kind="ExternalInput").ap()
    out_d = nc.dram_tensor("outT", [FOUT, N], F32, kind="ExternalOutput").ap()

    from contextlib import ExitStack
    with tile.TileContext(nc) as tc:
        with tc.tile_pool(name="const", bufs=1) as cpool, \
             tc.tile_pool(name="work", bufs=4) as wpool, \
             tc.tile_pool(name="adj", bufs=5) as apool:
            prep_ctx = ExitStack()
            pst = prep_ctx.enter_context(tc.tile_pool(name="pst", bufs=2, space="PSUM"))

            # ---- load constants / inputs (small tensors first, same queue) ----
            wt0 = cpool.tile([P, FOUT], F32, tag="wt0")
            wt1 = cpool.tile([P, FOUT], F32, tag="wt1")
            nc.sync.dma_start(wt0[:], wt_d[0:P, :])
            nc.sync.dma_start(wt1[:], wt_d[P:FIN, :])
            bcol = cpool.tile([FOUT, 1], F32, tag="bcol")
            nc.sync.dma_start(bcol[:], bcol_d)
            a1rep = cpool.tile([FOUT, P], F32, tag="a1rep")
            nc.sync.dma_start(a1rep[:], a1rep_d)
            a2rep = cpool.tile([P, FOUT], F32, tag="a2rep")
            nc.sync.dma_start(a2rep[:], a2rep_d)
            xt0 = cpool.tile([P, N], F32, tag="xt0")
            xt1 = cpool.tile([P, N], F32, tag="xt1")
            for c in range(NC4):
                sl = slice(c * 512, (c + 1) * 512)
                nc.sync.dma_start(xt0[:, sl], xT_d[0:P, sl])
                nc.sync.dma_start(xt1[:, sl], xT_d[P:FIN, sl])

            ident = cpool.tile([P, P], F32, tag="ident")
            make_identity(nc, ident[:])
            ones_col_f = cpool.tile([P, 1], F32, tag="ones_col_f")
            nc.gpsimd.memset(ones_col_f[:], 1.0)
            ones_col = cpool.tile([P, 1], F32R, tag="ones_col")
            nc.vector.tensor_copy(ones_col[:], ones_col_f[:])
            ones_row = cpool.tile([1, P], F32, tag="ones_row")
            nc.gpsimd.memset(ones_row[:], 1.0)

            # ---- hT[o, n] = W x + b  (fp32 matmuls, bias fused in ACT copy) ----
            hT = cpool.tile([FOUT, N], F32, tag="hT")
            for c in range(NC4):
                hps = pst.tile([FOUT, 512], F32, tag="tmp")
                sl = slice(c * 512, (c + 1) * 512)
                nc.tensor.matmul(hps[:], wt0[:], xt0[:, sl], start=True, stop=False)
                nc.tensor.matmul(hps[:], wt1[:], xt1[:, sl], start=False, stop=True)
                nc.scalar.activation(hT[:, sl], hps[:], AF.Identity, bias=bcol[:])

            # ---- s1b[p, i] = a1 . h[i] FIRST (loop's em depends only on this) ----
            s1b = cpool.tile([P, N], F32, tag="s1b")
            for c in range(NC4):
                bps = pst.tile([P, 512], F32, tag="tmp")
                sl = slice(c * 512, (c + 1) * 512)
                nc.tensor.matmul(bps[:], a1rep[:], hT[:, sl], start=True, stop=True)
                nc.scalar.activation(s1b[:, sl], bps[:], AF.Identity)

            # ---- h_nat[t] = hT[:, t].T via PE transpose; s2 per tile right after ----
            h_nat = []
            s2_cols = cpool.tile([P, NT], F32, tag="s2_cols")
            for t in range(NT):
                tps = pst.tile([P, P], F32, tag="tmp")
                nc.tensor.transpose(tps[:], hT[:, t * P:(t + 1) * P], ident[:])
                hn = cpool.tile([P, P], F32R, tag=f"h_nat{t}")
                nc.vector.tensor_copy(hn[:], tps[:])
                h_nat.append(hn)
                s2tmp = wpool.tile([P, FOUT], F32, tag="s2tmp")
                nc.vector.tensor_tensor(s2tmp[:], hn[:].bitcast(F32), a2rep[:], ALU.mult)
                nc.vector.reduce_sum(s2_cols[:, t:t + 1], s2tmp[:], axis=mybir.AxisListType.X)

            # ---- psum accumulators for h'T and S ----
            prep_ctx.close()
            acc_ctx = ExitStack()
            psacc = acc_ctx.enter_context(tc.tile_pool(name="psacc", bufs=1, space="PSUM"))
            sv_ctx = ExitStack()
            pssv = sv_ctx.enter_context(tc.tile_pool(name="pssv", bufs=1, space="PSUM"))
            acc = [psacc.tile([FOUT, 512], F32, tag=f"acc{c}", name=f"acc{c}") for c in range(NC4)]
            svec = [pssv.tile([1, 512], F32, tag=f"svec{c}", name=f"svec{c}") for c in range(NC4)]

            # ---- main j-loop ----
            for t in range(NT):
                adjm_t = apool.tile([P, N], FP8, tag="adjm")
                nc.gpsimd.dma_start(adjm_t[:], adjm_d[t * P:(t + 1) * P, :])

                s2c = s2_cols[:, t:t + 1]
                if t in GPS_EM_TILES:
                    # mask-add on gpsimd, s2 bias folded into ACT Prelu
                    em2 = wpool.tile([P, N], F32, tag="em")
                    nc.gpsimd.tensor_tensor(em2[:], s1b[:], adjm_t[:], ALU.add)
                    l_t = wpool.tile([P, N], F32, tag="lt")
                    nc.scalar.activation(l_t[:], em2[:], AF.Prelu, bias=s2c,
                                         scale=1.0, alpha=ALPHA)
                elif t in DVE_LEAKY_TILES:
                    em = wpool.tile([P, N], F32, tag="em")
                    nc.vector.scalar_tensor_tensor(em[:], in0=s1b[:], scalar=s2c,
                                                   in1=adjm_t[:], op0=ALU.add, op1=ALU.add)
                    l_t = wpool.tile([P, N], F32, tag="lt")
                    nc.vector.scalar_tensor_tensor(l_t[:], in0=em[:], scalar=ALPHA,
                                                   in1=em[:], op0=ALU.mult, op1=ALU.max)
                else:
                    em = wpool.tile([P, N], F32, tag="em")
                    nc.vector.tensor_tensor(em[:], s1b[:], adjm_t[:], ALU.add)
                    l_t = wpool.tile([P, N], F32, tag="lt")
                    nc.scalar.activation(l_t[:], em[:], AF.Prelu, bias=s2c,
                                         scale=1.0, alpha=ALPHA)
                p_t = wpool.tile([P, N], F32R, tag="pt")
                nc.scalar.activation(p_t[:], l_t[:], AF.Exp)

                first, last = (t == 0), (t == NT - 1)
                groups = [(svec, ones_col[:]), (acc, None)] if last else                          [(acc, None), (svec, ones_col[:])]
                for tiles, lhs in groups:
                    for c in range(NC4):
                        sl = slice(c * 512, (c + 1) * 512)
                        nc.tensor.matmul(tiles[c][:],
                                         lhs if lhs is not None else h_nat[t][:],
                                         p_t[:, sl], start=first, stop=last)

            # ---- tail: normalize + elu ----
            s_row = cpool.tile([1, N], F32, tag="s_row")
            for c in range(NC4):
                nc.vector.tensor_copy(s_row[:, c * 512:(c + 1) * 512], svec[c][:])
            sv_ctx.close()
            # column-shuffle so reciprocal runs wide: sv_cols[p, c*4+t] = S[c*512 + p*4 + t]
            sv_cols = cpool.tile([P, 4 * NC4], F32, tag="sv_cols")
            for c in range(NC4):
                nc.gpsimd.dma_start(sv_cols[:, c * 4:(c + 1) * 4], s_row[0:1, c * 512:(c + 1) * 512])
            rs_cols = cpool.tile([P, 4 * NC4], F32R, tag="rs_cols")
            with nc.allow_low_precision(reason="f32r for broadcast matmul"):
                nc.vector.reciprocal(rs_cols[:], sv_cols[:])
            # un-shuffle with the inverse DMA mapping
            rs_row = cpool.tile([1, N], F32R, tag="rs_row")
            for c in range(NC4):
                nc.gpsimd.dma_start(rs_row[0:1, c * 512:(c + 1) * 512].bitcast(F32),
                                  rs_cols[:, c * 4:(c + 1) * 4].bitcast(F32))

            tail_ctx = ExitStack()
            pstail = tail_ctx.enter_context(tc.tile_pool(name="pstail", bufs=2, space="PSUM"))
            ones_row_r = cpool.tile([1, P], F32R, tag="ones_row_r")
            nc.vector.tensor_copy(ones_row_r[:], ones_row[:])
            rb = cpool.tile([P, N], F32, tag="rb")
            hn_sb = cpool.tile([FOUT, N], F32, tag="hn_sb")
            m0 = cpool.tile([FOUT, N], F32, tag="m0")
            ex = cpool.tile([FOUT, N], F32, tag="ex")
            outT = cpool.tile([FOUT, N], F32, tag="outT")
            for c in range(NC4):
                rps = pstail.tile([P, 512], F32, tag="rps")
                sl = slice(c * 512, (c + 1) * 512)
                nc.tensor.matmul(rps[:], ones_row_r[:], rs_row[0:1, sl], start=True, stop=True)
                nc.scalar.activation(rb[:, sl], rps[:], AF.Identity)
                nc.vector.tensor_tensor(hn_sb[:, sl], acc[c][:], rb[:, sl], ALU.mult)
                nc.vector.tensor_scalar(m0[:, sl], hn_sb[:, sl], 0.0, None, op0=ALU.min)
                nc.scalar.activation(ex[:, sl], m0[:, sl], AF.Exp)
                nc.vector.scalar_tensor_tensor(outT[:, sl], in0=ex[:, sl], scalar=1.0,
                                               in1=hn_sb[:, sl], op0=ALU.subtract, op1=ALU.max)
                nc.sync.dma_start(out_d[:, sl], outT[:, sl])
            tail_ctx.close()
            acc_ctx.close()

    nc.compile()
    return nc


def make_in_maps(input, adj, W, b, a):
    x = np.asarray(input, dtype=np.float32)
    adj_np = np.asarray(adj)
    W_np = np.asarray(W, dtype=np.float32)
    b_np = np.asarray(b, dtype=np.float32)
    a_np = np.asarray(a, dtype=np.float32)

    xT = np.ascontiguousarray(x.transpose(0, 2, 1))                     # [B, FIN, N]
    adjT = adj_np.transpose(0, 2, 1)                                    # [B, N(j), N(i)]
    adjm = np.where(adjT > 0, 0.0, MASK_NEG).astype(ml_dtypes.float8_e4m3fn)
    adjm = np.ascontiguousarray(adjm)
    wt = np.ascontiguousarray(W_np.T)                                   # [FIN, FOUT]
    bcol = np.ascontiguousarray(b_np.reshape(FOUT, 1))
    a1rep = np.ascontiguousarray(np.broadcast_to(a_np[:FOUT, 0][:, None], (FOUT, P)))
    a2rep = np.ascontiguousarray(np.broadcast_to(a_np[FOUT:, 0][None, :], (P, FOUT)))

    return [{"xT": xT[c], "adjm": adjm[c], "wt": wt, "bcol": bcol,
             "a1rep": a1rep, "a2rep": a2rep} for c in range(B)]


def kernel(input, adj, W, b, a):
    from concourse.bass_utils import run_bass_kernel_spmd

    if "nc" not in _cache:
        _cache["nc"] = _build()
    nc = _cache["nc"]

    in_maps = make_in_maps(input, adj, W, b, a)
    res = run_bass_kernel_spmd(nc, in_maps, core_ids=list(range(B)))
    out = np.stack([np.asarray(res.results[c]["outT"]).T for c in range(B)])
    return np.ascontiguousarray(out, dtype=np.float32)



# revision 9
# speedup vs baseline: 1.8178x; 1.8178x over previous
"""Batched GAT layer (B=8, N=2048, Fin=256, Fout=128) on 8 Trainium2 NeuronCores.

Data-parallel over batch B — one batch element per core. Per core, a
factored-exponential formulation keeps the O(N^2) element-wise work on the
DVE fp16 fast paths and the softmax contraction on the PE with the
denominator fused in as a 129th output column:

  h        = x @ W.T + b                       (PE fp16, fp32 psum)
  s1[i]    = a1.h_i,  s2[j] = a2.h_j           (PE)
  exp(lrelu(s1+s2)) = max(e^{s1}e^{s2}, e^{.4 s1}e^{.4 s2})   (lrelu piecewise)
  u = e^{s1-c1}, u' = e^{.4 s1-c1'}            (ACT, replicated row tiles)
  v = e^{s2-c2}, v' = e^{.4 s2-c2'}            (ACT, per-partition columns)
  p[j,i]   = m[j,i] * max(u_i v_j, u'_i v'_j)  (DVE TS/TT; m = 0/1 mask)
  acc[i,:] = sum_j p[j,i] * [h_j | 1]          (PE, p-stationary, S = col 128)
  out      = elu(acc[:, :128] / acc[:, 128])   (DVE + ACT tail)

Some j-tiles (ALPHA_TILES) instead use the additive-mask path
(em -> Prelu -> Exp on ACT) to balance DVE/ACT load; the host encodes mask
rows per tile type (0/-60 additive vs 1/0 multiplicative, fp16). Shifts
c1,c2 (softmax-invariant) keep everything in fp16 range; calibrated
host-side from cheap score maxima, passed as per-partition bias vectors.

DMA: the mask (8MB fp16) is host-relayouted partition-contiguous in groups
of 4 j-tiles so each of the 4 DMAs moves 16KB-contiguous runs per
partition (big packets, hardware-dynamic queue) instead of 4KB rows.
"""
import numpy as np

B, N, FIN, FOUT = 8, 2048, 256, 128
P = 128
NT = N // P           # 16 n-tiles
NC4 = N // 512        # 4 chunks of 512
NG = 4                # mask DMA groups
GT = NT // NG         # tiles per group
ALPHA = 0.4
MASK_NEG = -60.0
MARGIN = 5.2

# j-tiles on the ACT additive-mask path (rest: factored DVE path).
# Spread across DMA groups so both engines stream smoothly.
ALPHA_TILES = frozenset({0, 2, 4, 6, 8, 10, 12, 14})

_cache = {}


def _build():
    import concourse.mybir as mybir
    import concourse.tile as tile
    from concourse import bacc
    from concourse.masks import make_identity
    from contextlib import ExitStack

    F32 = mybir.dt.float32
    F16 = mybir.dt.float16
    AF = mybir.ActivationFunctionType
    ALU = mybir.AluOpType

    nc = bacc.Bacc("TRN2", target_bir_lowering=False, debug=False)

    # xT relayouted [128, 2, 2048] partition-contiguous; mask in 4 groups of
    # 4 j-tiles, each [128, 4, 2048] partition-contiguous; out written
    # [128, 16, 128] partition-major (host de-permutes).
    xT_d = nc.dram_tensor("xTp", [P, 2 * N], F16, kind="ExternalInput").ap()
    madj_d = nc.dram_tensor("madjp", [NG * P, GT * N], F16, kind="ExternalInput").ap()
    wt_d = nc.dram_tensor("wt", [FIN, FOUT], F16, kind="ExternalInput").ap()
    brow_d = nc.dram_tensor("brow", [1, FOUT], F16, kind="ExternalInput").ap()
    a1rep_d = nc.dram_tensor("a1rep", [FOUT, P], F16, kind="ExternalInput").ap()
    a2col_d = nc.dram_tensor("a2col", [FOUT, 1], F16, kind="ExternalInput").ap()
    cvecs_d = nc.dram_tensor("cvecs", [P, 8], F32, kind="ExternalInput").ap()
    out_d = nc.dram_tensor("outp", [P, NT * FOUT], F16, kind="ExternalOutput").ap()

    with tile.TileContext(nc) as tc:
        with tc.tile_pool(name="const", bufs=1) as cpool, \
             tc.tile_pool(name="work", bufs=2) as wpool:
            # ---- small input DMAs (sync queue = hardware dynamic) ----
            wt0 = cpool.tile([P, FOUT], F16, tag="wt0")
            wt1 = cpool.tile([P, FOUT], F16, tag="wt1")
            brow = cpool.tile([1, FOUT], F16, tag="brow")
            a1rep = cpool.tile([FOUT, P], F16, tag="a1rep")
            a2col = cpool.tile([FOUT, 1], F16, tag="a2col")
            cvecs = cpool.tile([P, 8], F32, tag="cvecs")
            xt_all = cpool.tile([P, 2 * N], F16, tag="xt_all")
            nc.sync.dma_start(wt0[:], wt_d[0:P, :])
            nc.sync.dma_start(wt1[:], wt_d[P:FIN, :])
            nc.sync.dma_start(brow[:], brow_d)
            nc.sync.dma_start(a1rep[:], a1rep_d)
            nc.sync.dma_start(a2col[:], a2col_d)
            nc.sync.dma_start(cvecs[:], cvecs_d)
            nc.sync.dma_start(xt_all[:], xT_d)
            xt0 = xt_all[:, 0:N]
            xt1 = xt_all[:, N:2 * N]

            # mask: 4 partition-contiguous group DMAs on the sync queue
            adjm_all = cpool.tile([P, NT * N], F16, tag="adjm_all")
            for g in range(NG):
                nc.sync.dma_start(adjm_all[:, g * GT * N:(g + 1) * GT * N],
                                  madj_d[g * P:(g + 1) * P, :])

            def adjm(t):
                return adjm_all[:, t * N:(t + 1) * N]

            # constants built on gpsimd
            ident = cpool.tile([P, P], F16, tag="ident")
            make_identity(nc, ident[:])
            ones512 = cpool.tile([1, 512], F16, tag="ones512")
            nc.gpsimd.memset(ones512[:], 1.0)
            zcol = cpool.tile([1, P], F16, tag="zcol")
            nc.gpsimd.memset(zcol[:], 0.0)
            h_aug = []
            for t in range(NT):
                ha = cpool.tile([P, P + 4], F16, tag=f"haug{t}")
                nc.gpsimd.memset(ha[:, P:P + 1], 1.0)
                h_aug.append(ha)

            prep_ctx = ExitStack()
            pst = prep_ctx.enter_context(tc.tile_pool(name="pst", bufs=2, space="PSUM"))

            # ---- hT[o, n] = W x + b (bias via rank-1 matmul) ----
            hT = cpool.tile([FOUT, N], F16, tag="hT")
            for c in range(NC4):
                sl = slice(c * 512, (c + 1) * 512)
                hps = pst.tile([FOUT, 512], F32, tag="hps")
                nc.tensor.matmul(hps[:], wt0[:], xt0[:, sl], start=True, stop=False)
                nc.tensor.matmul(hps[:], wt1[:], xt1[:, sl], start=False, stop=False)
                nc.tensor.matmul(hps[:], brow[:], ones512[:], start=False, stop=True)
                nc.scalar.activation(hT[:, sl], hps[:], AF.Identity)

            # ---- s1b[p, i] = a1 . h_i (replicated across partitions) ----
            s1b = cpool.tile([P, N], F16, tag="s1b")
            for c in range(NC4):
                sl = slice(c * 512, (c + 1) * 512)
                bps = pst.tile([P, 512], F32, tag="bps")
                nc.tensor.matmul(bps[:], a1rep[:], hT[:, sl], start=True, stop=True)
                nc.scalar.activation(s1b[:, sl], bps[:], AF.Identity)

            # ---- s2 columns: s2cols[p, t] = a2 . h_{t*128+p} ----
            s2ps = pst.tile([P, NT], F32, tag="s2ps")
            for t in range(NT):
                nc.tensor.matmul(s2ps[:, t:t + 1], hT[:, t * P:(t + 1) * P],
                                 a2col[:], start=True, stop=True,
                                 skip_group_check=True)
            s2cols = cpool.tile([P, NT], F32, tag="s2cols")
            nc.vector.tensor_copy(s2cols[:], s2ps[:])

            # ---- v / v' columns (fp32) and u / u' reps (fp16) ----
            vcols = cpool.tile([P, NT], F32, tag="vcols")
            nc.scalar.activation(vcols[:], s2cols[:], AF.Exp, bias=cvecs[:, 2:3])
            vpcols = cpool.tile([P, NT], F32, tag="vpcols")
            nc.scalar.activation(vpcols[:], s2cols[:], AF.Exp, bias=cvecs[:, 3:4],
                                 scale=ALPHA)
            urep = cpool.tile([P, N], F16, tag="urep")
            nc.scalar.activation(urep[:], s1b[:], AF.Exp, bias=cvecs[:, 0:1])
            uprep = cpool.tile([P, N], F16, tag="uprep")
            nc.scalar.activation(uprep[:], s1b[:], AF.Exp, bias=cvecs[:, 1:2],
                                 scale=ALPHA)

            # ---- h_aug tiles via PE transpose of hT ----
            for t in range(NT):
                tps = pst.tile([P, P], F16, tag="tps")
                nc.tensor.transpose(tps[:], hT[:, t * P:(t + 1) * P], ident[:])
                nc.vector.tensor_copy(h_aug[t][:, 0:P], tps[:])

            prep_ctx.close()

            # ---- PSUM accumulators: 16 slices packed 3-per-bank ----
            acc_ctx = ExitStack()
            psacc = acc_ctx.enter_context(
                tc.tile_pool(name="psacc", bufs=1, space="PSUM"))
            accb = [psacc.tile([P, 512], F32, tag=f"accb{k}", name=f"accb{k}")
                    for k in range(6)]
            # start+stop zero-fill each bank once; the 16 accumulation slices
            # then run accumulate-only (multiple open start-groups per bank
            # lose their staged first write).
            for k in range(6):
                nc.tensor.matmul(accb[k][:], zcol[:], ones512[:],
                                 start=True, stop=True, skip_group_check=True)

            def acc_ap(q, lo, hi):
                base = (q % 3) * 160
                return accb[q // 3][:, base + lo:base + hi]

            # ---- main loop over j-tiles ----
            for t in range(NT):
                if t in ALPHA_TILES:
                    em = wpool.tile([P, N], F16, tag="em")
                    nc.vector.tensor_tensor(em[:], adjm(t), s1b[:], ALU.add)
                    lt = wpool.tile([P, N], F16, tag="lt")
                    nc.scalar.activation(lt[:], em[:], AF.Prelu,
                                         bias=s2cols[:, t:t + 1],
                                         scale=1.0, alpha=ALPHA)
                    pt = wpool.tile([P, N], F16, tag="pt", bufs=4)
                    nc.scalar.activation(pt[:], lt[:], AF.Exp, bias=cvecs[:, 4:5])
                else:
                    t1a = wpool.tile([P, N], F16, tag="t1a")
                    nc.vector.tensor_scalar(t1a[:], urep[:], vcols[:, t:t + 1],
                                            None, op0=ALU.mult)
                    t2 = wpool.tile([P, N], F16, tag="t2")
                    nc.vector.tensor_scalar(t2[:], uprep[:], vpcols[:, t:t + 1],
                                            None, op0=ALU.mult)
                    mx = wpool.tile([P, N], F16, tag="lt")
                    nc.vector.tensor_tensor(mx[:], t1a[:], t2[:], ALU.max)
                    pt = wpool.tile([P, N], F16, tag="pt", bufs=4)
                    nc.vector.tensor_tensor(pt[:], mx[:], adjm(t), ALU.mult)

                last = t == NT - 1
                for q in range(NT):
                    nc.tensor.matmul(acc_ap(q, 0, 129), pt[:, q * P:(q + 1) * P],
                                     h_aug[t][:, 0:129], start=False, stop=last,
                                     skip_group_check=True)

            # ---- tail: normalize + elu, one wide pass ----
            hn = cpool.tile([P, N], F16, tag="hn")
            for q in range(NT):
                rsq = cpool.tile([P, 1], F32, tag=f"rs{q}")
                nc.vector.reciprocal(rsq[:], acc_ap(q, 128, 129))
                if q % 2 == 0:
                    nc.vector.tensor_scalar(hn[:, q * P:(q + 1) * P],
                                            acc_ap(q, 0, 128), rsq[:], None,
                                            op0=ALU.mult)
                else:
                    nc.scalar.activation(hn[:, q * P:(q + 1) * P],
                                         acc_ap(q, 0, 128), AF.Identity,
                                         scale=rsq[:])
            m0 = cpool.tile([P, N], F16, tag="m0")
            nc.vector.tensor_scalar(m0[:], hn[:], 0.0, None, op0=ALU.min)
            ex = cpool.tile([P, N], F16, tag="ex")
            nc.scalar.activation(ex[:], m0[:], AF.Exp)
            exm1 = cpool.tile([P, N], F16, tag="exm1")
            nc.vector.tensor_scalar(exm1[:], ex[:], 1.0, None, op0=ALU.subtract)
            ov = cpool.tile([P, NT * FOUT], F16, tag="ov")
            nc.vector.tensor_tensor(ov[:], exm1[:], hn[:], ALU.max)
            nc.sync.dma_start(out_d, ov[:])
            acc_ctx.close()

    nc.compile()
    return nc


def make_in_maps(input, adj, W, b, a):
    x = np.asarray(input, dtype=np.float32)
    adj_np = np.asarray(adj)
    W_np = np.asarray(W, dtype=np.float32)
    b_np = np.asarray(b, dtype=np.float32)
    a_np = np.asarray(a, dtype=np.float32)
    a1 = a_np[:FOUT, 0]
    a2 = a_np[FOUT:, 0]

    # score-range calibration (cheap host dot products, sets fp16 shifts)
    w1 = W_np.T @ a1
    w2 = W_np.T @ a2
    s1 = x @ w1 + float(b_np @ a1)        # [B, N]
    s2 = x @ w2 + float(b_np @ a2)
    c1 = np.float32(s1.max() - MARGIN)
    c2 = np.float32(s2.max() - MARGIN)
    C = np.float32(c1 + c2)
    c1p = np.float32(ALPHA) * c1
    c2p = np.float32(C - c1p)
    cvecs = np.zeros((P, 8), dtype=np.float32)
    cvecs[:, 0] = -c1
    cvecs[:, 1] = -c1p
    cvecs[:, 2] = -c2
    cvecs[:, 3] = -c2p
    cvecs[:, 4] = -C

    # xT partition-contiguous: xTp[p, h*N + i] = x[i, h*128 + p]
    xT = x.transpose(0, 2, 1).reshape(B, 2, P, N)           # [B, half, p, i]
    xTp = np.ascontiguousarray(xT.transpose(0, 2, 1, 3)     # [B, p, half, i]
                               .reshape(B, P, 2 * N)).astype(np.float16)
    wt = np.ascontiguousarray(W_np.T).astype(np.float16)
    brow = np.ascontiguousarray(b_np.reshape(1, FOUT)).astype(np.float16)
    a1rep = np.ascontiguousarray(
        np.broadcast_to(a1[:, None], (FOUT, P))).astype(np.float16)
    a2col = np.ascontiguousarray(a2.reshape(FOUT, 1)).astype(np.float16)

    # mask [B, j, i] per-tile-type encoding, then partition-contiguous
    # groups: madjp[g*128 + p, (t%4)*N + i] = enc(adj[i, (4g + t%4)*128 + p])
    adjT = adj_np.transpose(0, 2, 1)       # [B, j, i]
    madj = np.empty((B, N, N), dtype=np.float16)
    alpha_rows = np.zeros(N, dtype=bool)
    for t in ALPHA_TILES:
        alpha_rows[t * P:(t + 1) * P] = True
    madj[:, alpha_rows, :] = np.where(
        adjT[:, alpha_rows, :] > 0, 0.0, MASK_NEG).astype(np.float16)
    madj[:, ~alpha_rows, :] = (adjT[:, ~alpha_rows, :] > 0).astype(np.float16)
    mg = madj.reshape(B, NG, GT, P, N)                       # [B, g, tg, p, i]
    madjp = np.ascontiguousarray(mg.transpose(0, 1, 3, 2, 4)  # [B, g, p, tg, i]
                                 .reshape(B, NG * P, GT * N))

    return [{"xTp": xTp[c], "madjp": madjp[c], "wt": wt, "brow": brow,
             "a1rep": a1rep, "a2col": a2col, "cvecs": cvecs}
            for c in range(B)]


def kernel(input, adj, W, b, a):
    from concourse.bass_utils import run_bass_kernel_spmd

    if "nc" not in _cache:
        _cache["nc"] = _build()
    nc = _cache["nc"]

    in_maps = make_in_maps(input, adj, W, b, a)
    res = run_bass_kernel_spmd(nc, in_maps, core_ids=list(range(B)))
    # outp[p, q*128 + o] -> out[q*128 + p, o]
    out = np.stack([
        np.asarray(res.results[c]["outp"]).reshape(P, NT, FOUT)
        .transpose(1, 0, 2).reshape(N, FOUT)
        for c in range(B)
    ])
    return np.ascontiguousarray(out.astype(np.float32))
